# revision 38
# baseline (speedup 1.0000x reference)
"""Trainium2 Bass kernel for single-head attention with QKV projections.

Problem: q,k,v [4, 2048, 1024] fp32; w_q/w_k/w_v [1024, 1024]; b_* [1024];
additive causal mask [1, 2048, 2048].
  query = q @ w_q.T + b_q ; key = k @ w_k.T + b_k ; value = v @ w_v.T + b_v
  att = softmax(query @ key.T / sqrt(D) + mask) ; out = att @ value

Weight-folding (host, exact algebra):
  query @ key.T = q M k.T + alpha_q 1.T + 1 beta_k.T + c
  with M = w_q.T @ w_k (host GEMM over weights only), beta = k @ (w_k.T b_q)
  (a host matvec over k), and alpha/c constant per q-row so they cancel in
  softmax.  Likewise out = softmax @ (v w_v.T + b_v) = (softmax @ v) w_v.T
  + b_v since softmax rows sum to one.  So the device only runs:
    qM = q @ M                              (proj, 1024 q rows/core)
    zT = (qM k.T).T + beta + causal mask    (transposed scores, k on parts)
    pT = exp(zT / sqrt(D)); l = colsum(pT)  (softmax, unnormalized)
    okdT[d, q] = sum_k v[k, d] pT[k, q]     (PV against RAW v)
    outT[e, q] = sum_d w_v[e, d] okdT[d, q] (folded V projection)
  Host divides by l, adds b_v, transposes back.  K and V projections never
  run on device: 3 GEMMs + 2 attention GEMMs become 2 proj + 2 attention.

Sharding: 8 cores = 4 batches x 2 members. Member m of a pair takes the 8
q-row tiles {m, m+2, ..., m+14} (odd/even interleave), which balances causal
work exactly: both members process attention groups of 256 q rows against
k-prefixes of length (512, 1024, 1536, 2048) — a uniform SPMD program.
Fully-masked k blocks are never computed (causal skipping); the 4 partially
masked k-blocks per group get an additive mask form (host-built per member).

Everything in bf16 on the PE (1 cycle/row at any free size), fp32 PSUM.
"""

import math

import numpy as np

import concourse.bass as bass
import concourse.mybir as mybir
import concourse.tile as tile
from concourse import bacc
from concourse.bass_utils import run_bass_kernel_spmd

B, S, D = 4, 2048, 1024
P = 128               # partitions
NDB = D // P          # 8 feature blocks
NKB = S // P          # 16 key blocks of 128
SQ = S // 2           # q rows per core
NT = SQ // P          # 8 q tiles per core
NG = 4                # attention groups of 256 q rows
GQ = 2 * P            # 256 q rows per group
KC = 512              # chunk for qM projection
SCALE = 1.0 / math.sqrt(D)
NEG = -3.2e10         # -1e9 * sqrt(D): masked logits (pre-exp-scale)

F32 = mybir.dt.float32
BF16 = mybir.dt.bfloat16


def _L(g):
    # k blocks (of 128) needed by group g: covers global tiles 4g+{0..3}
    return 4 * (g + 1)


def build_bass():
    nc = bacc.Bacc("TRN2", target_bir_lowering=False, debug=False, num_devices=8)

    # Activations blocked so the contraction dim (d for qM/QK, k for PV)
    # lands on SBUF partitions; every DMA contiguous per partition.
    # All inputs partition-major: one DMA each with large per-partition
    # contiguous runs (the HWDGE queues are packet-rate bound at ~2KB packets)
    qT = nc.dram_tensor("qT", [SQ // KC, P, NDB, KC], BF16, kind="ExternalInput")
    kT = nc.dram_tensor("kT", [S // KC, P, NDB, KC], BF16, kind="ExternalInput")
    vN = nc.dram_tensor("vN", [P, NKB, D], BF16, kind="ExternalInput")
    mT = nc.dram_tensor("mT", [P, NDB, NDB, P], BF16, kind="ExternalInput")
    wvT = nc.dram_tensor("wvT", [P, NDB, NDB, P], BF16, kind="ExternalInput")
    betaS = nc.dram_tensor("betaS", [P, NKB], F32, kind="ExternalInput")
    maskS = nc.dram_tensor("maskS", [P, 4, GQ], F32, kind="ExternalInput")
    out = nc.dram_tensor("out", [D, SQ], F32, kind="ExternalOutput")
    # unnormalized row sums, still split over the 128 k-partitions; host sums
    l_out = nc.dram_tensor("l_out", [P, NG, GQ], F32, kind="ExternalOutput")

    with tile.TileContext(nc) as tc:
        with (
            tc.tile_pool(name="const", bufs=1) as const_pool,
            tc.tile_pool(name="resid", bufs=1) as resid_pool,
        ):
            beta_sb = const_pool.tile([P, NKB], F32, tag="beta")
            nc.gpsimd.dma_start(out=beta_sb, in_=betaS[:, :])
            mask_sb = const_pool.tile([P, 4, GQ], F32, tag="mask")
            nc.gpsimd.dma_start(out=mask_sb, in_=maskS[:, :, :])

            # Residents: qMT (16KB/part), kT (32KB), vN (32KB), wv+mw (32KB)
            qMT_sb = resid_pool.tile([P, NDB, SQ], BF16, tag="qMT")
            kT_sb = resid_pool.tile([P, NDB, S], BF16, tag="kT")
            vN_sb = resid_pool.tile([P, NKB, D], BF16, tag="vN")
            wv_sb = resid_pool.tile([P, NDB, NDB, P], BF16, tag="wv")
            mw_sb = resid_pool.tile([P, NDB, NDB, P], BF16, tag="mw")

            # ---- Phase 1: qM projection -> qMT_sb [d', q] (transposed).
            # Queue split: m0 + qT on sync (needed first); M panels on the
            # scalar HWDGE queue; kT on the vector SWDGE queue; vN/wv
            # (needed later, by PV/FIN) behind the small consts on gpsimd.
            with (
                tc.tile_pool(name="wrm", bufs=1) as wrm_pool,
                tc.tile_pool(name="qin", bufs=2) as qin_pool,
                tc.tile_pool(name="ps1", bufs=6, space="PSUM") as ps1,
            ):
                # HAM warmup: ~32 matmuls on zeroed tiles, no DMA deps, so the
                # PE clock-gate opens during the DMA-bound kernel prologue and
                # the real matmuls start warm (2.4GHz) instead of cold (1.2).
                wrm_s = wrm_pool.tile([P, P], BF16, tag="wrms")
                wrm_m = wrm_pool.tile([P, KC], BF16, tag="wrmm")
                nc.vector.memset(wrm_s, 0)
                nc.vector.memset(wrm_m, 0)
                ps_w = ps1.tile([P, KC], F32, name="ps_w", tag="ps")
                NWARM = 32
                for i in range(NWARM):
                    nc.tensor.matmul(
                        ps_w, wrm_s, wrm_m,
                        start=(i == 0), stop=(i == NWARM - 1),
                    )

                n_sc = SQ // KC
                # M weights staged: eb 0 first (gates the first matmul),
                # then the rest while eb 0 computes
                nc.sync.dma_start(out=mw_sb[:, 0:1, :, :], in_=mT[:, 0:1, :, :])
                xs = []
                for sc in range(n_sc):
                    x_t = qin_pool.tile([P, NDB, KC], BF16, tag="qin", name="x_t")
                    nc.sync.dma_start(out=x_t[:, :NDB // 2, :], in_=qT[sc, :, :NDB // 2, :])
                    nc.sync.dma_start(out=x_t[:, NDB // 2:, :], in_=qT[sc, :, NDB // 2:, :])
                    xs.append(x_t)
                nc.sync.dma_start(out=mw_sb[:, 1:4, :, :], in_=mT[:, 1:4, :, :])
                nc.sync.dma_start(out=mw_sb[:, 4:, :, :], in_=mT[:, 4:, :, :])
                for c in range(S // KC):
                    nc.sync.dma_start(out=kT_sb[:, :, c * KC:(c + 1) * KC], in_=kT[c])
                nc.gpsimd.dma_start(out=vN_sb, in_=vN[:, :, :])
                nc.gpsimd.dma_start(out=wv_sb, in_=wvT[:, :, :, :])
                for eb in range(NDB):
                    pss = [ps1.tile([P, KC], F32, name="ps", tag="ps") for _ in range(n_sc)]
                    if eb == 0:
                        order = [(sc, db) for sc in range(n_sc) for db in range(NDB)]
                    else:
                        order = [(sc, db) for db in range(NDB) for sc in range(n_sc)]
                    for sc, db in order:
                        nc.tensor.matmul(
                            pss[sc],
                            mw_sb[:, eb, db, :],
                            xs[sc][:, db, :],
                            start=(db == 0),
                            stop=(db == NDB - 1),
                        )
                    # split the two evictions across ACT and DVE
                    nc.scalar.copy(out=qMT_sb[:, eb, 0:KC], in_=pss[0])
                    nc.vector.tensor_scalar_mul(
                        out=qMT_sb[:, eb, KC:2 * KC], in0=pss[1], scalar1=1.0
                    )

            # ---- Phase 2: attention, software-pipelined per 256-q group.
            with (
                tc.tile_pool(name="p", bufs=2) as p_pool,
                tc.tile_pool(name="okd", bufs=2) as okd_pool,
                tc.tile_pool(name="osb", bufs=2) as out_pool,
                tc.tile_pool(name="lac", bufs=2) as lac_pool,
                tc.tile_pool(name="psz", bufs=2, space="PSUM") as psz,
                tc.tile_pool(name="psv", bufs=4, space="PSUM") as psv,
                tc.tile_pool(name="pso", bufs=2, space="PSUM") as pso,
            ):
                def emit_qk(g):
                    L = _L(g)
                    # fixed max size so the pool ring reuses one allocation
                    p_g = p_pool.tile([P, NKB, GQ], BF16, tag="p", name=f"p{g}")
                    for kb in range(L):
                        ps_z = psz.tile([P, GQ], F32, tag="psz", name="ps_z")
                        for db in range(NDB):
                            nc.tensor.matmul(
                                ps_z,
                                kT_sb[:, db, kb * P:(kb + 1) * P],
                                qMT_sb[:, db, g * GQ:(g + 1) * GQ],
                                start=(db == 0),
                                stop=(db == NDB - 1),
                            )
                        r = kb - 4 * g
                        if r >= 0:
                            nc.vector.tensor_add(
                                out=ps_z, in0=ps_z, in1=mask_sb[:, r, :]
                            )
                        nc.scalar.activation(
                            out=p_g[:, kb, :],
                            in_=ps_z,
                            func=mybir.ActivationFunctionType.Exp,
                            bias=beta_sb[:, kb:kb + 1],
                            scale=SCALE,
                        )
                    return p_g

                def emit_pv(g, p_g):
                    L = _L(g)
                    # PSUM banks are 2KB: pack two 256-wide db outputs per bank
                    ps_v = [
                        psv.tile([P, 2 * GQ], F32, tag="psv", name="ps_v")
                        for _ in range(NDB // 2)
                    ]
                    # l = colsum(p) on DVE (partition-split; host finishes)
                    l_acc = lac_pool.tile([P, GQ], F32, tag="lac", name=f"lac{g}")
                    for kb in range(L):
                        if kb == 0:
                            nc.vector.tensor_scalar_mul(
                                out=l_acc, in0=p_g[:, 0, :], scalar1=1.0
                            )
                        else:
                            nc.vector.tensor_add(
                                out=l_acc, in0=l_acc, in1=p_g[:, kb, :]
                            )
                    # db outer / kb inner: a start=True clears has_written for
                    # the WHOLE bank, so the two half-bank accumulation groups
                    # sharing a bank must run sequentially, not interleaved.
                    # Evict each bank (both halves) as soon as the PE moves on
                    # to the next bank, alternating ACT/DVE; never read a bank
                    # the PE is still writing.
                    okd_sb = okd_pool.tile([P, NDB, GQ], BF16, tag="okd", name=f"okd{g}")
                    for db in range(NDB):
                        for kb in range(L):
                            nc.tensor.matmul(
                                ps_v[db // 2][:, (db % 2) * GQ:(db % 2 + 1) * GQ],
                                vN_sb[:, kb, db * P:(db + 1) * P],
                                p_g[:, kb, :],
                                start=(kb == 0),
                                stop=(kb == L - 1),
                            )
                        if db % 2 == 1:
                            j = db // 2
                            for h in range(2):
                                src = ps_v[j][:, h * GQ:(h + 1) * GQ]
                                dst = okd_sb[:, 2 * j + h, :]
                                if j % 2 == 0:
                                    nc.scalar.copy(out=dst, in_=src)
                                else:
                                    nc.vector.tensor_scalar_mul(
                                        out=dst, in0=src, scalar1=1.0
                                    )
                    nc.sync.dma_start(out=l_out[:, g, :], in_=l_acc)
                    return okd_sb

                def emit_fin(g, okd_sb):
                    # one [128, 8, 256] staging tile -> single DMA per group
                    o_all = out_pool.tile([P, NDB, GQ], F32, tag="osb", name=f"o{g}")
                    for ebp in range(NDB // 2):
                        ps_o = pso.tile([P, 2 * GQ], F32, tag="pso", name="ps_o")
                        for half in range(2):
                            eb = 2 * ebp + half
                            for db in range(NDB):
                                nc.tensor.matmul(
                                    ps_o[:, half * GQ:(half + 1) * GQ],
                                    wv_sb[:, eb, db, :],
                                    okd_sb[:, db, :],
                                    start=(db == 0),
                                    stop=(db == NDB - 1),
                                )
                        # evict both halves only after the PE left this bank
                        for half in range(2):
                            eb = 2 * ebp + half
                            src = ps_o[:, half * GQ:(half + 1) * GQ]
                            if ebp % 2 == 0:
                                nc.scalar.copy(out=o_all[:, eb, :], in_=src)
                            else:
                                nc.vector.tensor_scalar_mul(
                                    out=o_all[:, eb, :], in0=src, scalar1=1.0
                                )
                        # out[(eb*128+p), gGQ+c] <- o_all[p, eb, c]; per-pair
                        # DMAs so the tail transfer starts before FIN ends
                        dst = out[
                            2 * ebp * P:(2 * ebp + 2) * P, g * GQ:(g + 1) * GQ
                        ].rearrange("(eb p) c -> p eb c", p=P)
                        nc.sync.dma_start(
                            out=dst, in_=o_all[:, 2 * ebp:2 * ebp + 2, :]
                        )

                p0 = emit_qk(0)
                okd0 = emit_pv(0, p0)
                p1 = emit_qk(1)
                emit_fin(0, okd0)
                okd1 = emit_pv(1, p1)
                p2 = emit_qk(2)
                emit_fin(1, okd1)
                okd2 = emit_pv(2, p2)
                p3 = emit_qk(3)
                emit_fin(2, okd2)
                okd3 = emit_pv(3, p3)
                emit_fin(3, okd3)

    nc.finalize()
    return nc


_NC_CACHE = None
LAST_RESULT = None  # BassKernelResults from the most recent kernel() call


def _block_xT(x, chunk):
    """[s_total, D] activation -> [s_total/chunk, P, NDB, chunk] d-major blocks.

    Result[c, p, o, s] = x[c*chunk + s, o*P + p].
    """
    nchunk = x.shape[0] // chunk
    return np.ascontiguousarray(
        x.reshape(nchunk, chunk, NDB, P).transpose(0, 3, 2, 1)
    )


def _block_w_panels(wT, panel):
    """[D, D] pre-transposed weight -> [D/panel, P, NDB, panel] e-panels.

    Result[pan, p, o, e] = wT[o*P + p, pan*panel + e].
    """
    n = wT.shape[1] // panel
    return np.ascontiguousarray(
        wT.reshape(NDB, P, n, panel).transpose(2, 1, 0, 3)
    )


def _mask_forms(member):
    """[4, P, GQ] additive mask for the 4 trailing k-blocks of each group.

    Group g of member m covers global q tiles (4g+m, 4g+m+2) in its two
    128-col halves; k block 4g+r vs those tiles is below/diag/above causal.
    Form r is g-independent: pass iff (r - delta)*128 + i <= (c % 128) with
    delta = member + (0 if c < 128 else 2).
    """
    i = np.arange(P)[:, None]
    c = np.arange(GQ)[None, :]
    delta = np.where(c < P, member, member + 2)
    cmod = c % P
    forms = np.empty((4, P, GQ), dtype=np.float32)
    for r in range(4):
        passing = (r - delta) * P + i <= cmod
        forms[r] = np.where(passing, 0.0, NEG).astype(np.float32)
    return forms


def kernel(q, k, v, mask, w_q, b_q, w_k, b_k, w_v, b_v):
    global _NC_CACHE, LAST_RESULT
    import ml_dtypes

    bf16 = ml_dtypes.bfloat16
    f32 = np.float32

    if _NC_CACHE is None:
        _NC_CACHE = build_bass()
    nc = _NC_CACHE

    q = np.asarray(q, dtype=f32)
    k = np.asarray(k, dtype=f32)
    v = np.asarray(v, dtype=f32)
    w_q = np.asarray(w_q, dtype=f32)
    w_k = np.asarray(w_k, dtype=f32)
    w_v = np.asarray(w_v, dtype=f32)
    b_q = np.asarray(b_q, dtype=f32)
    b_k = np.asarray(b_k, dtype=f32)
    b_v = np.asarray(b_v, dtype=f32)

    # Folded weights (host, O(D^2) one-time): scores = qM.k + beta_k (+ terms
    # constant per q row, which softmax cancels).
    M = (w_q.T @ w_k).astype(f32)
    w_beta = (w_k.T @ b_q).astype(f32)
    # partition-major [p, eb, db, e] so each loads as one large-packet DMA
    mT = np.ascontiguousarray(
        _block_w_panels(M, P).transpose(1, 0, 2, 3)
    ).astype(bf16)
    wvT = np.ascontiguousarray(
        _block_w_panels(np.ascontiguousarray(w_v.T), P).transpose(1, 0, 2, 3)
    ).astype(bf16)

    masks = [
        np.ascontiguousarray(_mask_forms(m).transpose(1, 0, 2)) for m in range(2)
    ]
    tile_sel = [np.arange(m, S // P, 2) for m in range(2)]  # global tiles per member

    in_maps = []
    kT_b, vN_b, beta_b = {}, {}, {}
    for c in range(8):
        b, m = c // 2, c % 2
        if b not in kT_b:
            kT_b[b] = _block_xT(k[b].astype(bf16), KC)
            vN_b[b] = np.ascontiguousarray(
                v[b].astype(bf16).reshape(NKB, P, D).transpose(1, 0, 2)
            )
            beta = (k[b] @ w_beta) * SCALE
            beta_b[b] = np.ascontiguousarray(
                beta.reshape(NKB, P).T.astype(f32)
            )
        q_rows = q[b].reshape(S // P, P, D)[tile_sel[m]].reshape(SQ, D)
        in_maps.append({
            "qT": _block_xT(q_rows.astype(bf16), KC),
            "kT": kT_b[b],
            "vN": vN_b[b],
            "mT": mT,
            "wvT": wvT,
            "betaS": beta_b[b],
            "maskS": masks[m],
        })

    try:
        res = run_bass_kernel_spmd(nc, in_maps, list(range(8)))
    except Exception:
        # Rare transient device fault; the runtime recovers on re-execution.
        import time
        time.sleep(2.0)
        res = run_bass_kernel_spmd(nc, in_maps, list(range(8)))
    LAST_RESULT = res

    out = np.empty((B, S, D), dtype=f32)
    for c in range(8):
        b, m = c // 2, c % 2
        oT = res.results[c]["out"]          # [D(e), SQ(q)]
        l = res.results[c]["l_out"].sum(axis=0).reshape(SQ)  # [P, NG, GQ] -> [SQ]
        o = oT.T / l[:, None] + b_v[None, :]
        out[b].reshape(S // P, P, D)[tile_sel[m]] = o.reshape(NT, P, D)
    return out


# revision 39
# speedup vs baseline: 1.0134x; 1.0134x over previous
"""Trainium2 Bass kernel for single-head attention with QKV projections.

Problem: q,k,v [4, 2048, 1024] fp32; w_q/w_k/w_v [1024, 1024]; b_* [1024];
additive causal mask [1, 2048, 2048].
  query = q @ w_q.T + b_q ; key = k @ w_k.T + b_k ; value = v @ w_v.T + b_v
  att = softmax(query @ key.T / sqrt(D) + mask) ; out = att @ value

Weight-folding (host, exact algebra):
  query @ key.T = q M k.T + alpha_q 1.T + 1 beta_k.T + c
  with M = w_q.T @ w_k (host GEMM over weights only), beta = k @ (w_k.T b_q)
  (a host matvec over k), and alpha/c constant per q-row so they cancel in
  softmax.  Likewise out = softmax @ (v w_v.T + b_v) = (softmax @ v) w_v.T
  + b_v since softmax rows sum to one.  So the device only runs:
    qM = q @ M                              (proj, 1024 q rows/core)
    zT = (qM k.T).T + beta + causal mask    (transposed scores, k on parts)
    pT = exp(zT / sqrt(D)); l = colsum(pT)  (softmax, unnormalized)
    okdT[d, q] = sum_k v[k, d] pT[k, q]     (PV against RAW v)
    outT[e, q] = sum_d w_v[e, d] okdT[d, q] (folded V projection)
  Host divides by l, adds b_v, transposes back.  K and V projections never
  run on device: 3 GEMMs + 2 attention GEMMs become 2 proj + 2 attention.

Sharding: 8 cores = 4 batches x 2 members. Member m of a pair takes the 8
q-row tiles {m, m+2, ..., m+14} (odd/even interleave), which balances causal
work exactly: both members process attention groups of 256 q rows against
k-prefixes of length (512, 1024, 1536, 2048) — a uniform SPMD program.
Fully-masked k blocks are never computed (causal skipping); the 4 partially
masked k-blocks per group get an additive mask form (host-built per member).

Everything in bf16 on the PE (1 cycle/row at any free size), fp32 PSUM.
"""

import math

import numpy as np

import concourse.bass as bass
import concourse.mybir as mybir
import concourse.tile as tile
from concourse import bacc
from concourse.bass_utils import run_bass_kernel_spmd

B, S, D = 4, 2048, 1024
P = 128               # partitions
NDB = D // P          # 8 feature blocks
NKB = S // P          # 16 key blocks of 128
SQ = S // 2           # q rows per core
NT = SQ // P          # 8 q tiles per core
NG = 4                # attention groups of 256 q rows
GQ = 2 * P            # 256 q rows per group
KC = 512              # chunk for qM projection
SCALE = 1.0 / math.sqrt(D)
NEG = -3.2e10         # -1e9 * sqrt(D): masked logits (pre-exp-scale)

F32 = mybir.dt.float32
BF16 = mybir.dt.bfloat16


def _L(g):
    # k blocks (of 128) needed by group g: covers global tiles 4g+{0..3}
    return 4 * (g + 1)


def build_bass():
    nc = bacc.Bacc("TRN2", target_bir_lowering=False, debug=False, num_devices=8)

    # Activations blocked so the contraction dim (d for qM/QK, k for PV)
    # lands on SBUF partitions; every DMA contiguous per partition.
    # All inputs partition-major: one DMA each with large per-partition
    # contiguous runs (the HWDGE queues are packet-rate bound at ~2KB packets)
    qT = nc.dram_tensor("qT", [SQ // KC, P, NDB, KC], BF16, kind="ExternalInput")
    kT = nc.dram_tensor("kT", [S // KC, P, NDB, KC], BF16, kind="ExternalInput")
    vN = nc.dram_tensor("vN", [P, NKB, D], BF16, kind="ExternalInput")
    mT = nc.dram_tensor("mT", [P, NDB, NDB, P], BF16, kind="ExternalInput")
    wvT = nc.dram_tensor("wvT", [P, NDB, NDB, P], BF16, kind="ExternalInput")
    betaS = nc.dram_tensor("betaS", [P, NKB], F32, kind="ExternalInput")
    maskS = nc.dram_tensor("maskS", [P, 4, GQ], F32, kind="ExternalInput")
    out = nc.dram_tensor("out", [D, SQ], F32, kind="ExternalOutput")
    # unnormalized row sums, still split over the 128 k-partitions; host sums
    l_out = nc.dram_tensor("l_out", [P, NG, GQ], F32, kind="ExternalOutput")

    with tile.TileContext(nc) as tc:
        with (
            tc.tile_pool(name="const", bufs=1) as const_pool,
            tc.tile_pool(name="resid", bufs=1) as resid_pool,
        ):
            beta_sb = const_pool.tile([P, NKB], F32, tag="beta")
            nc.gpsimd.dma_start(out=beta_sb, in_=betaS[:, :])
            mask_sb = const_pool.tile([P, 4, GQ], F32, tag="mask")
            nc.gpsimd.dma_start(out=mask_sb, in_=maskS[:, :, :])

            # Residents: qMT (16KB/part), kT (32KB), vN (32KB), wv+mw (32KB)
            qMT_sb = resid_pool.tile([P, NDB, SQ], BF16, tag="qMT")
            kT_sb = resid_pool.tile([P, NDB, S], BF16, tag="kT")
            vN_sb = resid_pool.tile([P, NKB, D], BF16, tag="vN")
            wv_sb = resid_pool.tile([P, NDB, NDB, P], BF16, tag="wv")
            mw_sb = resid_pool.tile([P, NDB, NDB, P], BF16, tag="mw")

            # ---- Phase 1: qM projection -> qMT_sb [d', q] (transposed).
            # Queue split: m0 + qT on sync (needed first); M panels on the
            # scalar HWDGE queue; kT on the vector SWDGE queue; vN/wv
            # (needed later, by PV/FIN) behind the small consts on gpsimd.
            with (
                tc.tile_pool(name="wrm", bufs=1) as wrm_pool,
                tc.tile_pool(name="qin", bufs=2) as qin_pool,
                tc.tile_pool(name="ps1", bufs=6, space="PSUM") as ps1,
            ):
                # HAM warmup: ~32 matmuls on zeroed tiles, no DMA deps, so the
                # PE clock-gate opens during the DMA-bound kernel prologue and
                # the real matmuls start warm (2.4GHz) instead of cold (1.2).
                wrm_s = wrm_pool.tile([P, P], BF16, tag="wrms")
                wrm_m = wrm_pool.tile([P, KC], BF16, tag="wrmm")
                nc.vector.memset(wrm_s, 0)
                nc.vector.memset(wrm_m, 0)
                ps_w = ps1.tile([P, KC], F32, name="ps_w", tag="ps")
                NWARM = 32
                for i in range(NWARM):
                    nc.tensor.matmul(
                        ps_w, wrm_s, wrm_m,
                        start=(i == 0), stop=(i == NWARM - 1),
                    )

                n_sc = SQ // KC
                # Early-DMA parallelism across the three queues:
                #   sync:   m[0], qT chunk0, m[1], qT chunk1, then kT
                #   scalar: m[2:5], m[5:8]  (weight slices for later waves)
                #   gpsimd: consts, vN, wv  (needed only by PV/FIN)
                nc.sync.dma_start(out=mw_sb[:, 0:1, :, :], in_=mT[:, 0:1, :, :])
                xs = []
                for sc in range(n_sc):
                    x_t = qin_pool.tile([P, NDB, KC], BF16, tag="qin", name="x_t")
                    nc.sync.dma_start(out=x_t[:, :NDB // 2, :], in_=qT[sc, :, :NDB // 2, :])
                    nc.sync.dma_start(out=x_t[:, NDB // 2:, :], in_=qT[sc, :, NDB // 2:, :])
                    xs.append(x_t)
                    if sc == 0:
                        nc.sync.dma_start(
                            out=mw_sb[:, 1:2, :, :], in_=mT[:, 1:2, :, :]
                        )
                nc.scalar.dma_start(out=mw_sb[:, 2:5, :, :], in_=mT[:, 2:5, :, :])
                nc.scalar.dma_start(out=mw_sb[:, 5:, :, :], in_=mT[:, 5:, :, :])
                for c in range(S // KC):
                    nc.sync.dma_start(out=kT_sb[:, :, c * KC:(c + 1) * KC], in_=kT[c])
                nc.gpsimd.dma_start(out=vN_sb, in_=vN[:, :, :])
                nc.gpsimd.dma_start(out=wv_sb, in_=wvT[:, :, :, :])
                for eb in range(NDB):
                    pss = [ps1.tile([P, KC], F32, name="ps", tag="ps") for _ in range(n_sc)]
                    if eb == 0:
                        order = [(sc, db) for sc in range(n_sc) for db in range(NDB)]
                    else:
                        order = [(sc, db) for db in range(NDB) for sc in range(n_sc)]
                    for sc, db in order:
                        nc.tensor.matmul(
                            pss[sc],
                            mw_sb[:, eb, db, :],
                            xs[sc][:, db, :],
                            start=(db == 0),
                            stop=(db == NDB - 1),
                        )
                    # split the two evictions across ACT and DVE
                    nc.scalar.copy(out=qMT_sb[:, eb, 0:KC], in_=pss[0])
                    nc.vector.tensor_scalar_mul(
                        out=qMT_sb[:, eb, KC:2 * KC], in0=pss[1], scalar1=1.0
                    )

            # ---- Phase 2: attention, software-pipelined per 256-q group.
            with (
                tc.tile_pool(name="p", bufs=2) as p_pool,
                tc.tile_pool(name="okd", bufs=2) as okd_pool,
                tc.tile_pool(name="osb", bufs=2) as out_pool,
                tc.tile_pool(name="lac", bufs=2) as lac_pool,
                tc.tile_pool(name="psz", bufs=2, space="PSUM") as psz,
                tc.tile_pool(name="psv", bufs=4, space="PSUM") as psv,
                tc.tile_pool(name="pso", bufs=2, space="PSUM") as pso,
            ):
                def emit_qk(g):
                    L = _L(g)
                    # fixed max size so the pool ring reuses one allocation
                    p_g = p_pool.tile([P, NKB, GQ], BF16, tag="p", name=f"p{g}")
                    for kb in range(L):
                        ps_z = psz.tile([P, GQ], F32, tag="psz", name="ps_z")
                        for db in range(NDB):
                            nc.tensor.matmul(
                                ps_z,
                                kT_sb[:, db, kb * P:(kb + 1) * P],
                                qMT_sb[:, db, g * GQ:(g + 1) * GQ],
                                start=(db == 0),
                                stop=(db == NDB - 1),
                            )
                        r = kb - 4 * g
                        if r >= 0:
                            nc.vector.tensor_add(
                                out=ps_z, in0=ps_z, in1=mask_sb[:, r, :]
                            )
                        nc.scalar.activation(
                            out=p_g[:, kb, :],
                            in_=ps_z,
                            func=mybir.ActivationFunctionType.Exp,
                            bias=beta_sb[:, kb:kb + 1],
                            scale=SCALE,
                        )
                    return p_g

                def emit_pv(g, p_g):
                    L = _L(g)
                    # PSUM banks are 2KB: pack two 256-wide db outputs per bank
                    ps_v = [
                        psv.tile([P, 2 * GQ], F32, tag="psv", name="ps_v")
                        for _ in range(NDB // 2)
                    ]
                    # l = colsum(p) on DVE (partition-split; host finishes)
                    l_acc = lac_pool.tile([P, GQ], F32, tag="lac", name=f"lac{g}")
                    for kb in range(L):
                        if kb == 0:
                            nc.vector.tensor_scalar_mul(
                                out=l_acc, in0=p_g[:, 0, :], scalar1=1.0
                            )
                        else:
                            nc.vector.tensor_add(
                                out=l_acc, in0=l_acc, in1=p_g[:, kb, :]
                            )
                    # db outer / kb inner: a start=True clears has_written for
                    # the WHOLE bank, so the two half-bank accumulation groups
                    # sharing a bank must run sequentially, not interleaved.
                    # Evict each bank (both halves) as soon as the PE moves on
                    # to the next bank, alternating ACT/DVE; never read a bank
                    # the PE is still writing.
                    okd_sb = okd_pool.tile([P, NDB, GQ], BF16, tag="okd", name=f"okd{g}")
                    for db in range(NDB):
                        for kb in range(L):
                            nc.tensor.matmul(
                                ps_v[db // 2][:, (db % 2) * GQ:(db % 2 + 1) * GQ],
                                vN_sb[:, kb, db * P:(db + 1) * P],
                                p_g[:, kb, :],
                                start=(kb == 0),
                                stop=(kb == L - 1),
                            )
                        if db % 2 == 1:
                            j = db // 2
                            for h in range(2):
                                src = ps_v[j][:, h * GQ:(h + 1) * GQ]
                                dst = okd_sb[:, 2 * j + h, :]
                                if j % 2 == 0:
                                    nc.scalar.copy(out=dst, in_=src)
                                else:
                                    nc.vector.tensor_scalar_mul(
                                        out=dst, in0=src, scalar1=1.0
                                    )
                    nc.sync.dma_start(out=l_out[:, g, :], in_=l_acc)
                    return okd_sb

                def emit_fin(g, okd_sb):
                    # one [128, 8, 256] staging tile -> single DMA per group
                    o_all = out_pool.tile([P, NDB, GQ], F32, tag="osb", name=f"o{g}")
                    for ebp in range(NDB // 2):
                        ps_o = pso.tile([P, 2 * GQ], F32, tag="pso", name="ps_o")
                        for half in range(2):
                            eb = 2 * ebp + half
                            for db in range(NDB):
                                nc.tensor.matmul(
                                    ps_o[:, half * GQ:(half + 1) * GQ],
                                    wv_sb[:, eb, db, :],
                                    okd_sb[:, db, :],
                                    start=(db == 0),
                                    stop=(db == NDB - 1),
                                )
                        # evict both halves only after the PE left this bank
                        for half in range(2):
                            eb = 2 * ebp + half
                            src = ps_o[:, half * GQ:(half + 1) * GQ]
                            if ebp % 2 == 0:
                                nc.scalar.copy(out=o_all[:, eb, :], in_=src)
                            else:
                                nc.vector.tensor_scalar_mul(
                                    out=o_all[:, eb, :], in0=src, scalar1=1.0
                                )
                        # out[(eb*128+p), gGQ+c] <- o_all[p, eb, c]; per-pair
                        # DMAs so the tail transfer starts before FIN ends
                        dst = out[
                            2 * ebp * P:(2 * ebp + 2) * P, g * GQ:(g + 1) * GQ
                        ].rearrange("(eb p) c -> p eb c", p=P)
                        nc.sync.dma_start(
                            out=dst, in_=o_all[:, 2 * ebp:2 * ebp + 2, :]
                        )

                p0 = emit_qk(0)
                okd0 = emit_pv(0, p0)
                p1 = emit_qk(1)
                emit_fin(0, okd0)
                okd1 = emit_pv(1, p1)
                p2 = emit_qk(2)
                emit_fin(1, okd1)
                okd2 = emit_pv(2, p2)
                p3 = emit_qk(3)
                emit_fin(2, okd2)
                okd3 = emit_pv(3, p3)
                emit_fin(3, okd3)

    nc.finalize()
    return nc


_NC_CACHE = None
LAST_RESULT = None  # BassKernelResults from the most recent kernel() call


def _block_xT(x, chunk):
    """[s_total, D] activation -> [s_total/chunk, P, NDB, chunk] d-major blocks.

    Result[c, p, o, s] = x[c*chunk + s, o*P + p].
    """
    nchunk = x.shape[0] // chunk
    return np.ascontiguousarray(
        x.reshape(nchunk, chunk, NDB, P).transpose(0, 3, 2, 1)
    )


def _block_w_panels(wT, panel):
    """[D, D] pre-transposed weight -> [D/panel, P, NDB, panel] e-panels.

    Result[pan, p, o, e] = wT[o*P + p, pan*panel + e].
    """
    n = wT.shape[1] // panel
    return np.ascontiguousarray(
        wT.reshape(NDB, P, n, panel).transpose(2, 1, 0, 3)
    )


def _mask_forms(member):
    """[4, P, GQ] additive mask for the 4 trailing k-blocks of each group.

    Group g of member m covers global q tiles (4g+m, 4g+m+2) in its two
    128-col halves; k block 4g+r vs those tiles is below/diag/above causal.
    Form r is g-independent: pass iff (r - delta)*128 + i <= (c % 128) with
    delta = member + (0 if c < 128 else 2).
    """
    i = np.arange(P)[:, None]
    c = np.arange(GQ)[None, :]
    delta = np.where(c < P, member, member + 2)
    cmod = c % P
    forms = np.empty((4, P, GQ), dtype=np.float32)
    for r in range(4):
        passing = (r - delta) * P + i <= cmod
        forms[r] = np.where(passing, 0.0, NEG).astype(np.float32)
    return forms


def kernel(q, k, v, mask, w_q, b_q, w_k, b_k, w_v, b_v):
    global _NC_CACHE, LAST_RESULT
    import ml_dtypes

    bf16 = ml_dtypes.bfloat16
    f32 = np.float32

    if _NC_CACHE is None:
        _NC_CACHE = build_bass()
    nc = _NC_CACHE

    q = np.asarray(q, dtype=f32)
    k = np.asarray(k, dtype=f32)
    v = np.asarray(v, dtype=f32)
    w_q = np.asarray(w_q, dtype=f32)
    w_k = np.asarray(w_k, dtype=f32)
    w_v = np.asarray(w_v, dtype=f32)
    b_q = np.asarray(b_q, dtype=f32)
    b_k = np.asarray(b_k, dtype=f32)
    b_v = np.asarray(b_v, dtype=f32)

    # Folded weights (host, O(D^2) one-time): scores = qM.k + beta_k (+ terms
    # constant per q row, which softmax cancels).
    M = (w_q.T @ w_k).astype(f32)
    w_beta = (w_k.T @ b_q).astype(f32)
    # partition-major [p, eb, db, e] so each loads as one large-packet DMA
    mT = np.ascontiguousarray(
        _block_w_panels(M, P).transpose(1, 0, 2, 3)
    ).astype(bf16)
    wvT = np.ascontiguousarray(
        _block_w_panels(np.ascontiguousarray(w_v.T), P).transpose(1, 0, 2, 3)
    ).astype(bf16)

    masks = [
        np.ascontiguousarray(_mask_forms(m).transpose(1, 0, 2)) for m in range(2)
    ]
    tile_sel = [np.arange(m, S // P, 2) for m in range(2)]  # global tiles per member

    in_maps = []
    kT_b, vN_b, beta_b = {}, {}, {}
    for c in range(8):
        b, m = c // 2, c % 2
        if b not in kT_b:
            kT_b[b] = _block_xT(k[b].astype(bf16), KC)
            vN_b[b] = np.ascontiguousarray(
                v[b].astype(bf16).reshape(NKB, P, D).transpose(1, 0, 2)
            )
            beta = (k[b] @ w_beta) * SCALE
            beta_b[b] = np.ascontiguousarray(
                beta.reshape(NKB, P).T.astype(f32)
            )
        q_rows = q[b].reshape(S // P, P, D)[tile_sel[m]].reshape(SQ, D)
        in_maps.append({
            "qT": _block_xT(q_rows.astype(bf16), KC),
            "kT": kT_b[b],
            "vN": vN_b[b],
            "mT": mT,
            "wvT": wvT,
            "betaS": beta_b[b],
            "maskS": masks[m],
        })

    try:
        res = run_bass_kernel_spmd(nc, in_maps, list(range(8)))
    except Exception:
        # Rare transient device fault; the runtime recovers on re-execution.
        import time
        time.sleep(2.0)
        res = run_bass_kernel_spmd(nc, in_maps, list(range(8)))
    LAST_RESULT = res

    out = np.empty((B, S, D), dtype=f32)
    for c in range(8):
        b, m = c // 2, c % 2
        oT = res.results[c]["out"]          # [D(e), SQ(q)]
        l = res.results[c]["l_out"].sum(axis=0).reshape(SQ)  # [P, NG, GQ] -> [SQ]
        o = oT.T / l[:, None] + b_v[None, :]
        out[b].reshape(S // P, P, D)[tile_sel[m]] = o.reshape(NT, P, D)
    return out
